# revision 54
# baseline (speedup 1.0000x reference)
"""GQA cross-attention kernel for 8 trn2 NeuronCores — v3.

Problem: q [2, 2048, 32, 128] fp32, kv [2, 2048, 2, 8, 128] fp32
         -> softmax(q @ k^T / sqrt(128)) @ v  -> [2, 2048, 32, 128]

Sharding: 64 (batch, head) units over 8 cores: core c gets batch c//4,
q-heads [8*(c%4), 8*(c%4)+8) and kv-heads [2*(c%4), 2*(c%4)+2).

v3 design (~260us vs v2's 297us).  The PE floor is MM1+MM2 alone:
1024 matmuls x 213ns = 219us; everything else is kept off its path:
  - exp split between ACT (~78% of windows) and DVE (rest): a custom
    8-stage DVE op (EXP2BF16) computes bf16 bit patterns of
    lambda*2^(x/128) via magic-constant floor rounding + a deg-2
    mantissa polynomial, writing int16 (bitcast bf16).  The host
    pre-scales k by 128*log2e/sqrt(D) so no input multiply is needed;
    ACT windows use exp with scale=ln2/128, bias=log(lambda) to carry
    the same constant factor, which cancels in O/l.
  - l (exp-sum) on DVE as ~6 window-granular bf16 adds per block into
    accW[128,1536]; the [128,1536] partial goes to HBM whole and the
    HOST does the partition+slice reduction.  (The PE ones-matmul
    alternative costs ~93us of PE; GpSimd is useless here: its Q7
    PartitionAllReduce is 3.5us/call and its SBUF traffic slows
    concurrent DVE ops 1.7x.)
  - O^T PSUM is evacuated by the slack ACT engine (Copy), then DMA'd.

Per-core stream of 512 "k-steps" (32 blocks x 16 k-tiles; block =
(head, 512-wide q block)), MM1 3 steps ahead per exp window (WIN=3),
MM2 lagged LAG=18 steps so exp-engine jitter never stalls the PE.
PSUM: wpsum 2x3 banks + opsum 2 banks = 8.  Steady state runs the PE
at ~97% within its span (matmuls at 216ns).
"""

import math

import numpy as np

import concourse.bass as bass
import concourse.mybir as mybir
import concourse.tile as tile
from concourse import bacc
from concourse import dve_ops
from concourse.bass_utils import run_bass_kernel_spmd
from concourse.dve_ops import DveOp
from concourse.dve_spec import C0, C1, C2, C3, Spec, Src0, _spill_c3_to_src1, lower
from concourse.dve_uop import DveOpSpec

F32 = mybir.dt.float32
BF16 = mybir.dt.bfloat16
I16 = mybir.dt.int16
EXP = mybir.ActivationFunctionType.Exp

B, SQ, SK, H, HKV, D = 2, 2048, 2048, 32, 8, 128
N_CORES = 8
H_PER_CORE = H * B // N_CORES  # 8
KV_PER_CORE = HKV * B // N_CORES  # 2
SCALE = 1.0 / math.sqrt(D)
SQ_BLK = 512
WIN = 3  # k-steps per exp window -> [128, WIN*512] exp instruction
LAG = 18  # steps between MM1 emission and MM2 emission

# --- EXP2BF16 custom DVE op ------------------------------------------------
# input x = 128*log2e*SCALE*score (host pre-scales k); output int16 bits of
# bf16 ~= lam * 2^(x/128).  Calibrated for round-to-nearest int16 convert
# (HW-verified).
A2 = -0.002722233029857727  # quad coeff (imm2)
B1 = 1.0000477422221925  # lin coeff (via Src1 latch)
LAM_LOG2 = -63.46832458936395
EXP_C0 = 8128.0  # 128*64 - 64: exponent bias E0=64 + floor(-64)
EXP_C1 = 1.5 * 2**30  # magic: fp32 ulp 128 -> keeps multiples of 128
KPRE = 128.0 * math.log2(math.e) * SCALE  # host k pre-scale
ACT_SCALE = math.log(2.0) / 128.0
ACT_BIAS = LAM_LOG2 * math.log(2.0)

# engine split knobs: windows w%DVE_WIN_MOD in DVE_WIN_RES are exp'd on
# the DVE (custom op); the rest on ACT.
DVE_WIN_MOD = 9
DVE_WIN_RES = (4, 8)
# GpSimd is kept OFF the l path: its Q7 SBUF traffic slows concurrent DVE
# ops ~1.7x (403->678ns adds), a net loss.

_t = Src0 + C0
_u = _t + C1
_r0 = _u - C1
_fB = _t - _r0
_EXP_BODY = _r0 + (_fB * C2 + C3) * _fB


def _exp_reference(in0, in1, s0, s1, imm2):
    f32 = np.float32
    t = (in0.astype(f32) + f32(s0)).astype(f32)
    u = (t + f32(s1)).astype(f32)
    r0 = (u - f32(s1)).astype(f32)
    fB = (t - r0).astype(f32)
    w = ((fB * f32(imm2)).astype(f32) + f32(in1)).astype(f32)
    return (r0 + (w * fB).astype(f32)).astype(f32)


_EXP_SPEC = Spec(body=_spill_c3_to_src1(_EXP_BODY), reference=_exp_reference)


def _register_exp_op():
    for op in dve_ops.OPS:
        if op.name == "EXP2BF16":
            return op
    row = dve_ops._CUSTOM_DVE_ROW_BASE + len(dve_ops.OPS)
    assert row < 0x20
    dve_ops._SUB_OPCODE_FOR_NAME["EXP2BF16"] = row
    shas = {}
    for ver in ("v3", "v4"):
        try:
            uops = lower(_EXP_SPEC, ver=ver)
            shas[ver] = DveOpSpec(
                name="EXP2BF16", opcode=row, uops=uops, rd1_en=True
            ).sha(ver)
        except Exception:
            pass
    op = DveOp("EXP2BF16", _EXP_SPEC, subdim=False, uops_sha=shas)
    dve_ops.OPS.append(op)
    dve_ops.CUSTOM_DVE_SPECS["EXP2BF16"] = _EXP_SPEC
    return op


def build_nc(n_heads=H_PER_CORE, n_kv=KV_PER_CORE, sq=SQ, sk=SK):
    """Build the SPMD Bass program (identical on all cores)."""
    exp_op = _register_exp_op()
    heads_per_kv = n_heads // n_kv  # 4
    sk_tiles = sk // 128  # 16
    sq_blocks = sq // SQ_BLK  # 4
    n_blocks = n_heads * sq_blocks  # 32
    n_steps = n_blocks * sk_tiles  # 512
    n_wins = (n_steps + WIN - 1) // WIN

    nc = bacc.Bacc("TRN2", target_bir_lowering=False, debug=False)

    qT = nc.dram_tensor("qT", [n_heads, D, sq], BF16, kind="ExternalInput")
    kT = nc.dram_tensor("kT", [n_kv, D, sk], BF16, kind="ExternalInput")
    vt = nc.dram_tensor("vt", [n_kv, 128, sk_tiles * D], BF16, kind="ExternalInput")
    oT = nc.dram_tensor("oT", [n_heads, D, sq], F32, kind="ExternalOutput")
    # per-block l partials, reduced over partitions on the HOST (the Q7
    # PartitionAllReduce is ~3.5us per call — far too slow)
    lp = nc.dram_tensor(
        "lp", [n_heads, sq_blocks, 128, WIN * SQ_BLK], BF16, kind="ExternalOutput"
    )

    with tile.TileContext(nc) as tc:
        with (
            tc.tile_pool(name="inp", bufs=1) as inp_pool,
            tc.tile_pool(name="ppool", bufs=16) as ppool,
            tc.tile_pool(name="lacc", bufs=6) as lacc_pool,
            tc.tile_pool(name="outp", bufs=4) as outp,
            tc.tile_pool(name="wpsum", bufs=2, space="PSUM") as wpsum,
            tc.tile_pool(name="opsum", bufs=2, space="PSUM") as opsum,
        ):
            b1_sb = inp_pool.tile([128, 1], F32, tag="b1", name="b1_sb")
            nc.vector.memset(b1_sb[:], B1)
            bias_sb = inp_pool.tile([128, 1], F32, tag="bias", name="bias_sb")
            nc.vector.memset(bias_sb[:], ACT_BIAS)
            # Dummy exp to trigger the ACT table-set load (~2.7us) during
            # the DMA ramp instead of before the first real exp.
            warm_sb = inp_pool.tile([128, 1], BF16, tag="warm", name="warm_sb")
            warm_in = inp_pool.tile([128, 1], BF16, tag="warmi", name="warm_in")
            nc.vector.memset(warm_in[:], 1.0)
            nc.scalar.activation(
                warm_sb[:], warm_in[:], EXP, scale=ACT_SCALE, bias=bias_sb[:]
            )

            q_sb = [
                inp_pool.tile([D, sq], BF16, tag=f"q{h}", name=f"q_sb{h}")
                for h in range(n_heads)
            ]
            k_sb = [
                inp_pool.tile([D, sk], BF16, tag=f"k{g}", name=f"k_sb{g}")
                for g in range(n_kv)
            ]
            v_sb = [
                inp_pool.tile([128, sk_tiles * D], BF16, tag=f"v{g}", name=f"v_sb{g}")
                for g in range(n_kv)
            ]

            def qsl(h, j):
                return q_sb[h][:, bass.ts(j, SQ_BLK)]

            def ksl(g, t):
                return k_sb[g][:, bass.ts(t, 128)]

            # First wave on the sync (HWDGE) ring, in need-order for block 0.
            # Each dma_start costs ~0.6us of sync-queue time regardless of
            # size, so chunks are 128 cols only for the very first tiles and
            # 512+ cols after.  Everything beyond (g=0 later heads, all of
            # g=1) is issued from the gpsimd SWDGE ring in parallel.
            def need_order_dma(dst, src, sizes):
                off = 0
                for csz in sizes:
                    nc.sync.dma_start(
                        dst[:, off : off + csz], src[:, off : off + csz]
                    )
                    off += csz

            # First k/q on the sync (HWDGE) ring; the next k tiles + first
            # v chunk go out on the gpsimd SWDGE ring IN PARALLEL, so the
            # PE's second window isn't gated by sync-ring issue serialization.
            # Input stream is ISSUE-RATE bound (~0.6us of ring time per
            # dma_start; transfers fan out across the 16 DMA engines), so:
            # tiny first chunks for the first window, the next k tiles and
            # first v chunk on the parallel gpsimd SWDGE ring, and BIG
            # consolidated chunks for everything later.
            nc.sync.dma_start(k_sb[0][:, 0:128], kT[0][:, 0:128])
            nc.sync.dma_start(q_sb[0][:, 0:512], qT[0][:, 0:512])
            nc.gpsimd.dma_start(k_sb[0][:, 128:256], kT[0][:, 128:256])
            nc.gpsimd.dma_start(k_sb[0][:, 256:512], kT[0][:, 256:512])
            nc.gpsimd.dma_start(v_sb[0][:, 0:512], vt[0][:, 0:512])
            need_order_dma(k_sb[0][:, 512:], kT[0][:, 512:], [512, 1024])
            nc.sync.dma_start(q_sb[0][:, 512:], qT[0][:, 512:])
            nc.sync.dma_start(v_sb[0][:, 512:], vt[0][:, 512:])
            for h in range(1, heads_per_kv):
                nc.sync.dma_start(q_sb[h][:], qT[h][:])
            for g in range(1, n_kv):
                nc.sync.dma_start(k_sb[g][:], kT[g][:])
                for hh in range(heads_per_kv):
                    h = g * heads_per_kv + hh
                    nc.gpsimd.dma_start(q_sb[h][:], qT[h][:])
                nc.sync.dma_start(v_sb[g][:], vt[g][:])

            p_of_win = [None] * n_wins
            wtile = None
            state = {"o_ps": None, "accA": None, "accB": None}

            def step_hjt(s):
                blk, t = divmod(s, sk_tiles)
                h, j = divmod(blk, sq_blocks)
                return blk, h, j, t

            def emit_mm2(d):
                blk, h, j, t = step_hjt(d)
                g = h // heads_per_kv
                w, c = divmod(d, WIN)
                if t == 0:
                    state["o_ps"] = opsum.tile(
                        [128, SQ_BLK], F32, tag="o", name="o_ps"
                    )
                o_ps = state["o_ps"]
                nc.tensor.matmul(
                    o_ps[:],
                    v_sb[g][:, bass.ts(t, 128)],
                    p_of_win[w][:, bass.ts(c, SQ_BLK)],
                    start=(t == 0),
                    stop=(t == sk_tiles - 1),
                    skip_group_check=True,
                )

            def emit_lacc(d):
                # l accumulation on DVE, window-granular: one [128, W*512]
                # instr per contiguous window run (W<=3) into accW[128,1536].
                # A leading partial run is stashed and added after the first
                # full run's copy initializes accW.  The three 512-slices of
                # accW are folded on the HOST (accW goes to HBM whole).
                # ~6 DVE instrs per block instead of 15.
                blk, h, j, t = step_hjt(d)
                w, c = divmod(d, WIN)
                run_end = c == WIN - 1 or t == sk_tiles - 1
                if not run_end:
                    return
                c0 = c - min(c, t)  # run start position within the window
                width = (c - c0 + 1) * SQ_BLK
                off = c0 * SQ_BLK
                seg = p_of_win[w][:, off : off + width]
                if t <= WIN - 1 and width < WIN * SQ_BLK:
                    # leading partial run: defer until accW exists
                    state["stash"] = (seg, off, width)
                    return
                if t <= 2 * WIN - 1 and state.get("accW_blk") != blk:
                    # first full run: initialize accW
                    state["accW"] = lacc_pool.tile(
                        [128, WIN * SQ_BLK], BF16, tag="aW", name="accW"
                    )
                    state["accW_blk"] = blk
                    nc.vector.tensor_copy(state["accW"][:], seg)
                    if state.get("stash") is not None:
                        sseg, soff, swidth = state.pop("stash")
                        nc.vector.tensor_tensor(
                            state["accW"][:, soff : soff + swidth],
                            state["accW"][:, soff : soff + swidth],
                            sseg,
                            mybir.AluOpType.add,
                        )
                else:
                    accW = state["accW"]
                    nc.vector.tensor_tensor(
                        accW[:, off : off + width],
                        accW[:, off : off + width],
                        seg,
                        mybir.AluOpType.add,
                    )

            pend_ocopy = []

            def flush_ocopy():
                # Emit deferred O^T evacuations on ACT.  Called right before
                # a DVE-assigned exp window (ACT is idle then) and as a
                # safety valve, so the copy never delays an ACT exp.
                while pend_ocopy:
                    h, j, o_ps, last = pend_ocopy.pop(0)
                    ot_sb = outp.tile([128, SQ_BLK], F32, tag="ot", name="ot_sb")
                    nc.scalar.copy(ot_sb[:], o_ps[:])
                    if last:
                        for o in range(0, SQ_BLK, 128):
                            nc.sync.dma_start(
                                oT[h, :, j * SQ_BLK + o : j * SQ_BLK + o + 128],
                                ot_sb[:, o : o + 128],
                            )
                    else:
                        nc.sync.dma_start(oT[h, :, bass.ts(j, SQ_BLK)], ot_sb[:])

            def emit_block_tail(d):
                blk, h, j, t = step_hjt(d)
                last = blk == n_blocks - 1
                # l partial to HBM; host reduces partitions + window slices
                if last:  # split so the tail transfers parallelize
                    for o in range(0, WIN * SQ_BLK, SQ_BLK):
                        nc.sync.dma_start(
                            lp[h, j, :, o : o + SQ_BLK],
                            state["accW"][:, o : o + SQ_BLK],
                        )
                else:
                    nc.sync.dma_start(lp[h, j], state["accW"][:])
                pend_ocopy.append((h, j, state["o_ps"], last))
                if last or len(pend_ocopy) > 1:
                    flush_ocopy()

            for s in range(n_steps + LAG):
                d = s - LAG
                if s < n_steps:
                    blk, h, j, t = step_hjt(s)
                    g = h // heads_per_kv
                    w, c = divmod(s, WIN)
                    if c == 0:
                        wtile = wpsum.tile(
                            [128, WIN * SQ_BLK], F32, tag="w", name="w_ps"
                        )
                    nc.tensor.matmul(
                        wtile[:, bass.ts(c, SQ_BLK)],
                        ksl(g, t),
                        qsl(h, j),
                        start=True,
                        stop=True,
                    )
                    if c == WIN - 1 or s == n_steps - 1:
                        width = (c + 1) * SQ_BLK
                        ptile = ppool.tile(
                            [128, WIN * SQ_BLK], BF16, tag="p", name="p_sb"
                        )
                        # Startup: before MM2s begin, MM1 eats windows at 2x
                        # the steady rate — split exp 50/50 with the idle DVE.
                        # Steady state: give DVE the windows that END mid-block
                        # (t in 8..10), away from its block-tail l-work, so
                        # the DVE queue never delays those exps (3/16 share).
                        end_t = (3 * w + 2) % sk_tiles
                        dve_win = (
                            w % 2 == 1 if w < 12 else end_t in (8, 9, 10)
                        )
                        if dve_win:
                            flush_ocopy()
                        if dve_win:
                            nc.vector._custom_dve(
                                exp_op,
                                out=ptile[:, :width].bitcast(I16),
                                in0=wtile[:, :width],
                                in1=b1_sb[:],
                                s0=EXP_C0,
                                s1=EXP_C1,
                                imm2=A2,
                            )
                        else:
                            nc.scalar.activation(
                                ptile[:, :width], wtile[:, :width], EXP,
                                scale=ACT_SCALE, bias=bias_sb[:],
                            )
                        p_of_win[w] = ptile
                if d >= 0:
                    emit_mm2(d)
                    emit_lacc(d)
                    if d % sk_tiles == sk_tiles - 1:
                        emit_block_tail(d)

    nc.compile()
    return nc


_NC_CACHE = {}


def _get_nc():
    if "nc" not in _NC_CACHE:
        _NC_CACHE["nc"] = build_nc()
    return _NC_CACHE["nc"]


def make_in_maps(q, kv):
    import ml_dtypes

    q = np.asarray(q)
    kv = np.asarray(kv)
    k = kv[:, :, 0]  # [B, Sk, Hkv, D]
    v = kv[:, :, 1]  # [B, Sk, Hkv, D]
    qT_all = np.ascontiguousarray(
        q.transpose(0, 2, 3, 1).astype(ml_dtypes.bfloat16)
    )  # [B, H, D, Sq]
    kT_all = np.ascontiguousarray(
        (k * np.float32(KPRE)).transpose(0, 2, 3, 1).astype(ml_dtypes.bfloat16)
    )  # [B, Hkv, D, Sk], pre-scaled for the exp bit trick
    # vt[b, hkv, p, t, d] = v[b, t*128 + p, hkv, d]
    vt_all = np.ascontiguousarray(
        v.reshape(B, SK // 128, 128, HKV, D)
        .transpose(0, 3, 2, 1, 4)
        .astype(ml_dtypes.bfloat16)
    ).reshape(B, HKV, 128, (SK // 128) * D)

    in_maps = []
    for c in range(N_CORES):
        b = c // (N_CORES // B)
        part = c % (N_CORES // B)
        h0 = part * H_PER_CORE
        g0 = part * KV_PER_CORE
        in_maps.append(
            {
                "qT": qT_all[b, h0 : h0 + H_PER_CORE],
                "kT": kT_all[b, g0 : g0 + KV_PER_CORE],
                "vt": vt_all[b, g0 : g0 + KV_PER_CORE],
            }
        )
    return in_maps


def gather_output(results):
    out = np.empty((B, SQ, H, D), np.float32)
    for c in range(N_CORES):
        b = c // (N_CORES // B)
        part = c % (N_CORES // B)
        h0 = part * H_PER_CORE
        oTc = results[c]["oT"]  # [8, 128, 2048] unnormalized O^T
        lpc = results[c]["lp"]  # [8, 4, 128, 1536] bf16 exp-sum partials
        l = (
            lpc.astype(np.float32)
            .reshape(H_PER_CORE, SQ // SQ_BLK, 128, WIN, SQ_BLK)
            .sum(axis=(2, 3))
            .reshape(H_PER_CORE, SQ)
        )  # [8, 2048]
        o = oTc / l[:, None, :]
        out[b, :, h0 : h0 + H_PER_CORE, :] = o.transpose(2, 0, 1)
    return out


def run(q, kv, trace=False, **kwargs):
    nc = _get_nc()
    in_maps = make_in_maps(q, kv)
    last_err = None
    for _attempt in range(3):
        try:
            res = run_bass_kernel_spmd(
                nc, in_maps, core_ids=list(range(N_CORES)), trace=trace, **kwargs
            )
            return gather_output(res.results), res
        except Exception as e:  # transient NRT device wedge: retry
            last_err = e
            import time

            time.sleep(5)
    raise last_err


def kernel(q, kv):
    out, _ = run(q, kv, trace=False)
    return out
